# revision 10
# baseline (speedup 1.0000x reference)
"""Trainium2 Bass kernel for the dendritic-branch spiking FNN (DH_SFNN).

Model (per reference):
  branch_in = x @ W_in.T + b_in                  # (B,T,H*BR)
  per t:  i_d = beta*i_d + (1-beta)*branch_in_t  # beta = sigmoid(tau_n), (H,BR)
          v   = alpha*v + (1-alpha)*i_d.sum(br)  # alpha = sigmoid(tau_m), (H,)
          spike = (v >= 1); v -= spike; counts += spike
  out = counts @ W_out.T + b_out                 # (B,D_OUT)

Strategy: data-parallel over batch across 8 cores (32 rows each). On each
core: 2-pass fp16 GEMM (x split hi/lo vs fp16 W -- the W-quantization error
is below the spike-flip threshold) into a (br,j,h)-tiled layout, scaled by
(1-alpha)(1-beta) on the ACT engine so the scan output is directly the
per-branch drive w.  The dendrite IIR runs as one fused tensor_tensor_scan
per (m-tile, chunk); branches are combined in-place (uw_j += u23_j).  The
nonlinear spike loop keeps its state in the history buffer (op2 of step t
writes hist[t], op1 of t+1 reads it), 2 DVE ops per timestep.  Spike counts
are recovered via the telescoped identity

  sum_t s_t = (1-a) sum_t p_t + a*(p_T - p_0) + sum_t w_t

where sum_t p_t comes from a log-depth fold of hist and sum_t w_t from a
tiny extra GEMM against per-chunk time-summed inputs (Xbar = sum_t x) plus
carry boundary terms -- this removes the per-chunk is_le + strided reduce
passes entirely.  Readout is a small PE matmul against W_out.
"""

import sys

if "/opt/trn_rl_repo" not in sys.path:
    sys.path.insert(0, "/opt/trn_rl_repo")

from contextlib import ExitStack

import numpy as np

import concourse.bass as bass
import concourse.mybir as mybir
import concourse.tile as tile
from concourse import bacc

B, T, D_IN, H, BR, D_OUT = 256, 500, 700, 200, 2, 35
NCORES = 8
BL = B // NCORES          # local batch = 32
NK = 6                    # k-tiles; D_IN padded 700 -> 768 so every tile is 128
DP = NK * 128             # padded contraction dim (768)
M = 4                     # m-tiles, m=(br,j): h = (m%2)*128+p, br = m//2
OP = M * 128              # padded output rows (512)
NJ = 2                    # h groups (j=0: h<128, j=1: h 128..199)
NF = NJ * BL              # spike-loop columns (chains per partition)


def _f32(a):
    return np.ascontiguousarray(a, dtype=np.float32)


def _split16(a):
    hi = a.astype(np.float16)
    lo = (a - hi.astype(np.float32)).astype(np.float16)
    return np.stack([hi, lo])


def _build(T_, C_, alpha_u):
    """Build the single-core Bass program. alpha_u: python float (uniform
    soma decay -- guaranteed by the reference's tau_m = 2.0)."""
    NCH = T_ // C_
    BG = 8                 # batches per matmul n-group
    NG = BL // BG          # 4 n-groups
    NN = BG * C_           # matmul free dim
    NCB = NCH * BL         # ubar free dim (chunks x batch)
    assert NN <= 512 and T_ % C_ == 0
    fp32 = mybir.dt.float32
    fp16 = mybir.dt.float16
    AF = mybir.ActivationFunctionType
    AL = mybir.AluOpType

    nc = bacc.Bacc("TRN2", target_bir_lowering=False, debug=False,
                   num_devices=NCORES)

    xt_d = nc.dram_tensor("xt", [2, NCH, NG, 128, NK * BG * C_],
                          fp16, kind="ExternalInput")
    wt_d = nc.dram_tensor("wt", [NK, 128, OP], fp16, kind="ExternalInput")
    xbt_d = nc.dram_tensor("xbt", [2, 128, NK * NCB], fp16,
                           kind="ExternalInput")
    sc2_d = nc.dram_tensor("sc2", [128, M], fp32, kind="ExternalInput")
    b2_d = nc.dram_tensor("b2", [128, M], fp32, kind="ExternalInput")
    b2c_d = nc.dram_tensor("b2c", [128, M], fp32, kind="ExternalInput")
    bt_d = nc.dram_tensor("bt", [128, M], fp32, kind="ExternalInput")
    ivb_d = nc.dram_tensor("ivb", [128, M], fp32, kind="ExternalInput")
    d0_d = nc.dram_tensor("d0", [M, 128, BL * C_], fp32, kind="ExternalInput")
    woutT_d = nc.dram_tensor("woutT", [2 * 128, D_OUT], fp32, kind="ExternalInput")
    bout_d = nc.dram_tensor("bout", [D_OUT, 1], fp32, kind="ExternalInput")

    out_d = nc.dram_tensor("out", [D_OUT, BL], fp32, kind="ExternalOutput")
    # tiny passthrough tensor so benchmark harnesses can chain executions
    tok_d = nc.dram_tensor("tok", [1, 16], fp32, kind="ExternalInput")
    tok_o = nc.dram_tensor("tok_out", [1, 16], fp32, kind="ExternalOutput")

    with tile.TileContext(nc) as tc, ExitStack() as ctx:
        const = ctx.enter_context(tc.tile_pool(name="const", bufs=1))
        st = ctx.enter_context(tc.tile_pool(name="state", bufs=1))
        uwp = ctx.enter_context(tc.tile_pool(name="uw", bufs=2))
        u23p = ctx.enter_context(tc.tile_pool(name="u23", bufs=1))
        xp = ctx.enter_context(tc.tile_pool(name="xin", bufs=2))
        ps = ctx.enter_context(tc.tile_pool(name="psum", bufs=4, space="PSUM"))
        pso = ctx.enter_context(tc.tile_pool(name="psout", bufs=1, space="PSUM"))
        scr = ctx.enter_context(tc.tile_pool(name="scr", bufs=2))

        # ---- constants ----
        w_sb = const.tile([128, NK * OP], fp16, tag="wsb")
        nc.sync.dma_start(w_sb[:].rearrange("p (k o) -> p k o", k=NK),
                          wt_d.ap().rearrange("k p o -> p k o"))
        sc2 = const.tile([128, M], fp32)
        nc.sync.dma_start(sc2[:], sc2_d.ap())
        b2 = const.tile([128, M], fp32)
        nc.sync.dma_start(b2[:], b2_d.ap())
        b2c = const.tile([128, M], fp32)
        nc.sync.dma_start(b2c[:], b2c_d.ap())
        bt = const.tile([128, M], fp32)
        nc.sync.dma_start(bt[:], bt_d.ap())
        ivb = const.tile([128, M], fp32)
        nc.sync.dma_start(ivb[:], ivb_d.ap())
        d0_sb = const.tile([128, M * BL * C_], fp32, tag="d0")
        nc.sync.dma_start(d0_sb[:].rearrange("p (m x) -> p m x", m=M),
                          d0_d.ap().rearrange("m p x -> p m x"))
        woutT_sb = const.tile([128, 2 * D_OUT], fp32)
        nc.sync.dma_start(woutT_sb[:, 0:D_OUT], woutT_d.ap()[0:128])
        nc.sync.dma_start(woutT_sb[:, D_OUT:2 * D_OUT], woutT_d.ap()[128:256])
        bout_sb = const.tile([D_OUT, 1], fp32)
        nc.sync.dma_start(bout_sb[:], bout_d.ap())
        xb_sbs = []
        for s in range(2):
            xb_sb = const.tile([128, NK * NCB], fp16, tag=f"xb{s}")
            nc.sync.dma_start(xb_sb[:], xbt_d.ap()[s])
            xb_sbs.append(xb_sb)

        # ---- ubar GEMM: sum_t u'' per (m, chunk, b) ----
        ubar = const.tile([128, M * NCB], fp32, tag="ubar")
        for m in range(M):
            pu = pso.tile([128, NCB], fp32, tag="pu")
            i = 0
            for k in range(NK):
                for s in range(2):
                    nc.tensor.matmul(
                        pu[:],
                        w_sb[:, k * OP + m * 128:k * OP + (m + 1) * 128],
                        xb_sbs[s][:, k * NCB:(k + 1) * NCB],
                        start=(i == 0), stop=(i == 2 * NK - 1))
                    i += 1
            nc.scalar.activation(ubar[:, m * NCB:(m + 1) * NCB], pu[:],
                                 AF.Identity, bias=b2c[:, m:m + 1],
                                 scale=sc2[:, m:m + 1])

        # ---- state tiles ----
        counts = st.tile([128, NF], fp32, tag="cnt")
        pstate = st.tile([128, NF], fp32, tag="pst")
        tA = st.tile([128, NF], fp32, tag="tA")
        hist = [st.tile([128, C_ * NF], fp32, tag=f"hist{i}",
                        name=f"hist{i}") for i in range(2)]
        wti = [st.tile([128, C_ * NF], fp32, tag=f"wti{i}",
                       name=f"wti{i}") for i in range(2)]
        carry = [st.tile([128, M * BL], fp32, tag=f"carry{i}",
                         name=f"carry{i}") for i in range(2)]
        swt = st.tile([128, M * BL], fp32, tag="swt")
        wsum = st.tile([128, 2 * BL * C_], fp32, tag="wsum")
        u23 = u23p.tile([128, 2 * BL * C_], fp32, tag="u23")
        nc.vector.memset(counts[:], 0.0)
        nc.vector.memset(pstate[:], 0.0)
        nc.vector.memset(carry[1][:], 0.0)

        for c in range(NCH):
            cnew = carry[c % 2]
            cprev = carry[(c + 1) % 2]
            histc = hist[c % 2]
            wtic = wti[c % 2]
            uw = uwp.tile([128, 2 * BL * C_], fp32, tag="uw")

            # -- GEMM: u[m, b, t] = (xh + xl) @ W16, scaled ((1-a)(1-b)) --
            for g in range(NG):
                x_sbs = []
                for s in range(2):
                    x_sb = xp.tile([128, NK * NN], fp16, tag=f"xsb{s}")
                    nc.sync.dma_start(x_sb[:], xt_d.ap()[s, c, g])
                    x_sbs.append(x_sb)
                for m in range(M):
                    pt = ps.tile([128, NN], fp32, tag="pt")
                    i = 0
                    for k in range(NK):
                        for s in range(2):
                            nc.tensor.matmul(
                                pt[:],
                                w_sb[:, k * OP + m * 128:k * OP + (m + 1) * 128],
                                x_sbs[s][:, k * NN:(k + 1) * NN],
                                start=(i == 0), stop=(i == 2 * NK - 1))
                            i += 1
                    dst = uw if m < 2 else u23
                    mm = m if m < 2 else m - 2
                    nc.scalar.activation(
                        dst[:, mm * BL * C_ + g * NN:mm * BL * C_ + (g + 1) * NN],
                        pt[:], AF.Identity,
                        bias=b2[:, m:m + 1], scale=sc2[:, m:m + 1])

            # -- dendrite IIR: g = beta*g + u'', fused scan per m-tile --
            for m in range(M):
                dst = uw if m < 2 else u23
                mm = m if m < 2 else m - 2
                um = dst[:, mm * BL * C_:(mm + 1) * BL * C_]
                um3 = um.rearrange("p (b c) -> p b c", c=C_)
                if c > 0:
                    # u[:, b, 0] += beta * carry_b
                    nc.vector.scalar_tensor_tensor(
                        um3[:, :, 0], cprev[:, m * BL:(m + 1) * BL],
                        bt[:, m:m + 1], um3[:, :, 0], AL.mult, AL.add)
                nc.vector.tensor_tensor_scan(
                    um[:], d0_sb[:, m * BL * C_:(m + 1) * BL * C_], um[:],
                    0.0, AL.mult, AL.add)
                # capture end-of-chunk scan state (before branch-add)
                nc.scalar.copy(cnew[:, m * BL:(m + 1) * BL],
                               um3[:, :, C_ - 1])

            # -- branch combine (GPSIMD, frees DVE): w_j = g_j + g_{2+j};
            #    then ACT transposes b-major -> t-major wti for the spike loop
            wre = wtic[:].rearrange("p (c f) -> p f c", f=NF)
            for j in range(NJ):
                nc.gpsimd.tensor_tensor(
                    wsum[:, j * BL * C_:(j + 1) * BL * C_],
                    uw[:, j * BL * C_:(j + 1) * BL * C_],
                    u23[:, j * BL * C_:(j + 1) * BL * C_], AL.add)
                nc.scalar.copy(
                    wre[:, j * BL:(j + 1) * BL, :],
                    wsum[:, j * BL * C_:(j + 1) * BL * C_].rearrange(
                        "p (b c) -> p b c", c=C_))

            # -- sum_t w via ubar + carry algebra:
            #    sw_m = (ubar_m[c] + beta*(cprev - cnew)) / (1-beta) --
            for m in range(M):
                sl = slice(m * BL, (m + 1) * BL)
                nc.vector.tensor_tensor(swt[:, sl], cprev[:, sl], cnew[:, sl],
                                        AL.subtract)
                nc.vector.scalar_tensor_tensor(
                    swt[:, sl], swt[:, sl], bt[:, m:m + 1],
                    ubar[:, m * NCB + c * BL:m * NCB + (c + 1) * BL],
                    AL.mult, AL.add)
                nc.scalar.activation(swt[:, sl], swt[:, sl], AF.Identity,
                                     bias=0.0, scale=ivb[:, m:m + 1])
            for m in range(M):
                j = m % 2
                nc.vector.tensor_tensor(counts[:, j * BL:(j + 1) * BL],
                                        counts[:, j * BL:(j + 1) * BL],
                                        swt[:, m * BL:(m + 1) * BL], AL.add)

            # -- spike loop (state in hist; negated potential) --
            for t in range(C_):
                pprev = pstate[:] if t == 0 else histc[:, (t - 1) * NF:t * NF]
                ht = histc[:, t * NF:(t + 1) * NF]
                # tA = alpha*p - w_t
                nc.vector.scalar_tensor_tensor(
                    tA[:], pprev, float(alpha_u), wtic[:, t * NF:(t + 1) * NF],
                    AL.mult, AL.subtract)
                # p' = (tA <= -1) + tA
                nc.vector.scalar_tensor_tensor(
                    ht, tA[:], -1.0, tA[:], AL.is_le, AL.add)
            nc.scalar.copy(pstate[:], histc[:, (C_ - 1) * NF:C_ * NF])

            # -- sum_t p_t: log-depth fold of hist (GPSIMD), counts on DVE --
            nt = C_
            while nt > 1:
                h = nt // 2
                nc.gpsimd.tensor_tensor(
                    histc[:, 0:h * NF], histc[:, 0:h * NF],
                    histc[:, (nt - h) * NF:nt * NF], AL.add)
                nt -= h
            nc.vector.scalar_tensor_tensor(
                counts[:], histc[:, 0:NF], float(1.0 - alpha_u), counts[:],
                AL.mult, AL.add)

        # counts += alpha * p_final  (global telescoped boundary term)
        nc.vector.scalar_tensor_tensor(
            counts[:], pstate[:], float(alpha_u), counts[:], AL.mult, AL.add)

        # -- readout: out = W_out @ counts + b_out --
        po = pso.tile([D_OUT, BL], fp32, tag="po")
        nc.tensor.matmul(po[:], woutT_sb[:, 0:D_OUT], counts[:, 0:BL],
                         start=True, stop=False)
        nc.tensor.matmul(po[:], woutT_sb[0:H - 128, D_OUT:2 * D_OUT],
                         counts[0:H - 128, BL:2 * BL], start=False, stop=True)
        out_sb = scr.tile([D_OUT, BL], fp32, tag="osb")
        nc.scalar.activation(out_sb[:], po[:], AF.Identity,
                             bias=bout_sb[:, 0:1], scale=1.0)
        nc.sync.dma_start(out_d.ap(), out_sb[:])

        tok_sb = scr.tile([1, 16], fp32, tag="tok")
        nc.sync.dma_start(tok_sb[:], tok_d.ap())
        nc.sync.dma_start(tok_o.ap(), tok_sb[:])

    nc.compile()
    return nc


def _prep_host(x, W_in, b_in, tau_n, tau_m, W_out, b_out, T_, C_):
    """Host-side constant prep. Returns (shared_inputs, per_core_x, alpha)."""
    x = _f32(x); W_in = _f32(W_in); b_in = _f32(b_in)
    tau_n = _f32(tau_n); tau_m = _f32(tau_m)
    W_out = _f32(W_out); b_out = _f32(b_out)

    beta = _f32(1.0 / (1.0 + np.exp(-tau_n.astype(np.float64))))   # (H,BR)
    alpha = _f32(1.0 / (1.0 + np.exp(-tau_m.astype(np.float64))))  # (H,)
    assert np.all(alpha == alpha[0]), "kernel assumes uniform tau_m"
    alpha_u = float(alpha[0])
    one = np.float32(1.0)
    NCH = T_ // C_
    NCB = NCH * BL

    # m-tile map: m=(br,j) -> rows p: h = (m%2)*128+p, o = h*BR + br
    wt = np.zeros((NK, 128, OP), np.float32)
    sc2 = np.zeros((128, M), np.float32)
    b2 = np.zeros((128, M), np.float32)
    bt = np.zeros((128, M), np.float32)
    ivb = np.zeros((128, M), np.float32)
    for m in range(M):
        br, j = m // 2, m % 2
        for p in range(128):
            h = j * 128 + p
            if h >= H:
                continue
            o = h * BR + br
            sc2[p, m] = (one - alpha[h]) * (one - beta[h, br])
            b2[p, m] = sc2[p, m] * b_in[o]
            bt[p, m] = beta[h, br]
            ivb[p, m] = one / (one - beta[h, br])
            wrow = np.zeros(DP, np.float32)
            wrow[:D_IN] = W_in[o]
            wt[:, :, m * 128 + p] = wrow.reshape(NK, 128)
    b2c = np.float32(C_) * b2
    d0 = np.zeros((M, 128, BL * C_), np.float32)
    for m in range(M):
        d0[m, :, :] = bt[:, m:m + 1]
        d0[m, :, 0::C_] = 0.0
    woutT = np.zeros((256, D_OUT), np.float32)
    woutT[:H, :] = W_out.T
    bout = b_out.reshape(D_OUT, 1)

    wt16 = wt.astype(np.float16)
    shared = dict(wt=wt16, sc2=sc2, b2=b2, b2c=b2c, bt=bt, ivb=ivb,
                  d0=d0, woutT=_f32(woutT), bout=_f32(bout))
    xts = []
    xbts = []
    BG = 8
    for core in range(NCORES):
        xl_ = x[core * BL:(core + 1) * BL, :T_, :]       # (BL,T,D_IN)
        xp_ = np.zeros((BL, T_, DP), np.float32)
        xp_[:, :, :D_IN] = xl_
        # -> [c, g, p, (k,bi,t)]
        xt = xp_.reshape(BL // BG, BG, NCH, C_, NK, 128).transpose(
            2, 0, 4, 5, 1, 3)                            # (NCH,NG,NK,128,BG,C)
        # layout per (c,g): [128, (k, bi, t)]
        xt = xt.transpose(0, 1, 3, 2, 4, 5).reshape(NCH, BL // BG, 128,
                                                    NK * BG * C_)
        xts.append(_split16(_f32(xt)))
        # xbar: per-chunk time sums -> [s, 128, (k, cb)] with cb = c*BL + b
        xb = xp_.reshape(BL, NCH, C_, DP).sum(axis=2)    # (BL, NCH, DP)
        xb = xb.transpose(2, 1, 0).reshape(NK, 128, NCB)  # (k,128,(c,b))
        xb = xb.transpose(1, 0, 2).reshape(128, NK * NCB)
        xbts.append(_split16(_f32(xb)))
    return shared, xts, xbts, alpha_u


TRACE = False          # set by test harness for profiling runs
LAST_RESULT = None


def kernel(x, W_in, b_in, tau_n, tau_m, W_out, b_out):
    global LAST_RESULT
    from concourse.bass_utils import run_bass_kernel_spmd

    T_, C_ = T, 50
    shared, xts, xbts, alpha_u = _prep_host(
        x, W_in, b_in, tau_n, tau_m, W_out, b_out, T_, C_)
    nc = _build(T_, C_, alpha_u)
    tok = np.zeros((1, 16), np.float32)
    in_maps = [dict(shared, xt=xts[core], xbt=xbts[core], tok=tok)
               for core in range(NCORES)]
    res = run_bass_kernel_spmd(nc, in_maps, core_ids=list(range(NCORES)),
                               trace=TRACE)
    LAST_RESULT = res
    out = np.empty((B, D_OUT), np.float32)
    for core in range(NCORES):
        out[core * BL:(core + 1) * BL, :] = res.results[core]["out"].T
    return out
